# revision 25
# baseline (speedup 1.0000x reference)
"""AutoCorrelation kernel for Trainium2 (Bass/Tile), 8-core data parallel.

Math: the reference computes rfft over the zero-padded head dim (D=64 -> L=512),
multiplies conj(Q)*K, irffts, then MEANS over heads AND the whole lag axis.
Summing a circular correlation over all lags factorizes:
    sum_t corr[t] = (sum_d q[d]) * (sum_d k[d])
so  x_corr_mean[b,l] = 1/(H*L) * sum_h (sum_d q[b,l,h,:]) * (sum_d k[b,l,h,:]).
Then top-6 over l per batch, softmax, weighted sum of values rows -> [B,H,D].

Sharding: batch 16 -> 2 per core across 8 cores, no cross-core communication.
"""

import numpy as np

import concourse.bass as bass
import concourse.mybir as mybir
import concourse.tile as tile
from concourse.masks import make_identity
from concourse.bass_utils import run_bass_kernel_spmd

B, L, H, D = 16, 512, 8, 64
HD = H * D                  # 512
NCORES = 8
BPC = B // NCORES           # 2 batches per core
ROWS = BPC * L              # 1024 rows of [HD] per core
P = 128
NT = ROWS // P              # 8 chunks of 128 rows
TPB = L // P                # 4 chunks per batch
KTOP = 6                    # k = int(log(512)) = 6
NDMA = 2                    # main-load DMA splits per tensor (2x1MB; with the
                            # merge + store DMAs this fills all 8 HWDGE lanes
                            # exactly once -- lane reuse would add a second
                            # sync-wait, which HWDGE descriptors can't encode)
TPD = NT // NDMA            # chunks per DMA tile
SCALE = 1.0 / (H * L)

_CACHE = {}


def _emit(tc, q, k, v, out):
    # out: list of BPC separate [1, HD] DRAM APs (separate tensors avoid a
    # spurious WAW sync between the two row-store DMAs).
    nc = tc.nc
    from contextlib import ExitStack

    with ExitStack() as ctx:
        main = ctx.enter_context(tc.tile_pool(name="main", bufs=NDMA))
        small = ctx.enter_context(tc.tile_pool(name="small", bufs=1))
        psum = ctx.enter_context(tc.tile_pool(name="psum", bufs=1, space="PSUM"))

        ident = small.tile([P, P], mybir.dt.float32)
        make_identity(nc, ident[:])

        q3 = q.rearrange("(t p) m -> t p m", p=P)
        k3 = k.rearrange("(t p) m -> t p m", p=P)

        sq_all = small.tile([P, NT * H], mybir.dt.float32)
        sk_all = small.tile([P, NT * H], mybir.dt.float32)
        corr_all = small.tile([P, NT], mybir.dt.float32)

        # Row sums over D, then per-head product and sum over H.
        # corr_all[p, t] = sum_h sq*sk for row t*128+p (unscaled).
        for j in range(NDMA):
            t0 = j * TPD
            qt = main.tile([P, TPD, HD], mybir.dt.float32, tag="qt")
            kt = main.tile([P, TPD, HD], mybir.dt.float32, tag="kt")
            nc.sync.dma_start(
                out=qt[:], in_=q3[t0 : t0 + TPD].rearrange("t p m -> p t m")
            )
            nc.sync.dma_start(
                out=kt[:], in_=k3[t0 : t0 + TPD].rearrange("t p m -> p t m")
            )
            sq = sq_all[:, t0 * H : (t0 + TPD) * H]
            sk = sk_all[:, t0 * H : (t0 + TPD) * H]
            nc.vector.reduce_sum(
                out=sq,
                in_=qt[:].rearrange("p t (h d) -> p (t h) d", d=D),
                axis=mybir.AxisListType.X,
            )
            nc.vector.reduce_sum(
                out=sk,
                in_=kt[:].rearrange("p t (h d) -> p (t h) d", d=D),
                axis=mybir.AxisListType.X,
            )
            prod = main.tile([P, TPD * H], mybir.dt.float32, tag="prod")
            nc.vector.tensor_mul(prod[:], sq, sk)
            nc.vector.reduce_sum(
                out=corr_all[:, t0 : t0 + TPD],
                in_=prod[:].rearrange("p (t h) -> p t h", h=H),
                axis=mybir.AxisListType.X,
            )

        # [128, 8] -> [8, 128] via PE transpose, then two partition-sliced
        # DMAs rake it into [2, 512] (batch on partitions, position on free).
        psumT = psum.tile([NT, P], mybir.dt.float32)
        nc.tensor.transpose(out=psumT[:], in_=corr_all[:], identity=ident[:])
        corrT = small.tile([NT, P], mybir.dt.float32)
        nc.vector.tensor_copy(corrT[:], psumT[:])

        corr2 = small.tile([BPC, L], mybir.dt.float32)
        for b in range(BPC):
            nc.sync.dma_start(
                out=corr2[b : b + 1, :], in_=corrT[b * TPB : (b + 1) * TPB, :]
            )

        # Top-8 values + indices per batch row.
        maxv = small.tile([BPC, 8], mybir.dt.float32)
        maxi = small.tile([BPC, 8], mybir.dt.uint32)
        nc.vector.max(out=maxv[:], in_=corr2[:])
        nc.vector.max_index(out=maxi[:], in_max=maxv[:], in_values=corr2[:])

        # softmax over the top-6 of corr*SCALE
        negmax = small.tile([BPC, 1], mybir.dt.float32)
        nc.scalar.activation(
            out=negmax[:],
            in_=maxv[:, 0:1],
            func=mybir.ActivationFunctionType.Copy,
            scale=-SCALE,
        )
        e = small.tile([BPC, KTOP], mybir.dt.float32)
        nc.scalar.activation(
            out=e[:],
            in_=maxv[:, 0:KTOP],
            func=mybir.ActivationFunctionType.Exp,
            bias=negmax[:, 0:1],
            scale=SCALE,
        )
        s = small.tile([BPC, 1], mybir.dt.float32)
        nc.vector.reduce_sum(out=s[:], in_=e[:], axis=mybir.AxisListType.X)
        rs = small.tile([BPC, 1], mybir.dt.float32)
        nc.vector.reciprocal(out=rs[:], in_=s[:])
        w = small.tile([BPC, KTOP], mybir.dt.float32)
        nc.vector.tensor_scalar_mul(w[:], e[:], rs[:, 0:1])

        # Global row index (b*512 + l) as float, exact for < 2^24.
        idxf = small.tile([BPC, 8], mybir.dt.float32)
        nc.vector.tensor_copy(idxf[:], maxi[:])
        bofs_i = small.tile([BPC, 1], mybir.dt.int32)
        nc.gpsimd.iota(bofs_i[:], pattern=[[0, 1]], base=0, channel_multiplier=L)
        bofs = small.tile([BPC, 1], mybir.dt.float32)
        nc.vector.tensor_copy(bofs[:], bofs_i[:])
        nc.vector.tensor_scalar_add(idxf[:], idxf[:], bofs[:, 0:1])

        # Stream-transpose weights and indices from the free axis onto
        # partitions (engine APs must start at partition 0, so one 32x32
        # staging tile per quantity; batch becomes the column index).
        stage_w = small.tile([32, 32], mybir.dt.float32)
        nc.vector.memset(stage_w[:], 0.0)
        nc.vector.tensor_copy(stage_w[0:BPC, 0:KTOP], w[:])
        stageT_w = small.tile([32, 32], mybir.dt.float32)
        nc.vector.transpose(out=stageT_w[:], in_=stage_w[:])

        stage_i = small.tile([32, 32], mybir.dt.float32)
        nc.vector.memset(stage_i[:], 0.0)
        nc.vector.tensor_copy(stage_i[0:BPC, 0:KTOP], idxf[:, 0:KTOP])
        stageT_i = small.tile([32, 32], mybir.dt.float32)
        nc.vector.transpose(out=stageT_i[:], in_=stage_i[:])
        idxu = small.tile([32, BPC], mybir.dt.uint32)
        nc.vector.tensor_copy(idxu[:], stageT_i[:, 0:BPC])

        # Per batch: gather the 6 selected value rows (2KB each) from DRAM,
        # weighted-sum them with one tiny matmul (K=6, M=1, N=512).
        for b in range(BPC):
            gath = small.tile([KTOP, HD], mybir.dt.float32, tag=f"gath{b}")
            nc.gpsimd.indirect_dma_start(
                out=gath[:],
                out_offset=None,
                in_=v,
                in_offset=bass.IndirectOffsetOnAxis(
                    ap=idxu[0:KTOP, b : b + 1], axis=0
                ),
            )
            acc = psum.tile([1, HD], mybir.dt.float32, tag=f"acc{b}")
            nc.tensor.matmul(
                out=acc[:],
                lhsT=stageT_w[0:KTOP, b : b + 1],
                rhs=gath[:],
                start=True,
                stop=True,
            )
            outt = small.tile([1, HD], mybir.dt.float32, tag=f"outt{b}")
            nc.vector.tensor_copy(outt[:], acc[:])
            nc.sync.dma_start(out=out[b], in_=outt[:])


def _build_bass():
    import concourse.bacc as bacc

    nc = bacc.Bacc(trn_type="TRN2", target_bir_lowering=False, debug=False)
    q = nc.dram_tensor("q", [ROWS, HD], mybir.dt.float32, kind="ExternalInput").ap()
    k = nc.dram_tensor("k", [ROWS, HD], mybir.dt.float32, kind="ExternalInput").ap()
    v = nc.dram_tensor("v", [ROWS, HD], mybir.dt.float32, kind="ExternalInput").ap()
    outs = [
        nc.dram_tensor(
            f"out{b}", [1, HD], mybir.dt.float32, kind="ExternalOutput"
        ).ap()
        for b in range(BPC)
    ]
    with tile.TileContext(nc) as tc:
        _emit(tc, q, k, v, outs)
    nc.compile()
    return nc


def _get_nc():
    if "nc" not in _CACHE:
        _CACHE["nc"] = _build_bass()
    return _CACHE["nc"]


def run_sharded(queries, keys, values, trace=False, **kw):
    """Shard over 8 cores, run, gather. Returns (out [16,8,64], BassKernelResults)."""
    nc = _get_nc()
    q = np.ascontiguousarray(np.asarray(queries, dtype=np.float32))
    k = np.ascontiguousarray(np.asarray(keys, dtype=np.float32))
    v = np.ascontiguousarray(np.asarray(values, dtype=np.float32))
    in_maps = []
    for c in range(NCORES):
        sl = slice(c * BPC, (c + 1) * BPC)
        in_maps.append(
            {
                "q": q[sl].reshape(ROWS, HD),
                "k": k[sl].reshape(ROWS, HD),
                "v": v[sl].reshape(ROWS, HD),
            }
        )
    res = run_bass_kernel_spmd(nc, in_maps, list(range(NCORES)), trace=trace, **kw)
    out = np.empty((B, H, D), dtype=np.float32)
    for c in range(NCORES):
        for b in range(BPC):
            out[c * BPC + b] = res.results[c][f"out{b}"].reshape(H, D)
    return out, res


def kernel(queries, keys, values, B=None, **_ignored):
    out, _ = run_sharded(queries, keys, values, trace=False)
    return out


# revision 35
# speedup vs baseline: 1.0073x; 1.0073x over previous
"""AutoCorrelation kernel for Trainium2 (Bass/Tile), 8-core data parallel.

Math: the reference computes rfft over the zero-padded head dim (D=64 -> L=512),
multiplies conj(Q)*K, irffts, then MEANS over heads AND the whole lag axis.
Summing a circular correlation over all lags factorizes:
    sum_t corr[t] = (sum_d q[d]) * (sum_d k[d])
so  x_corr_mean[b,l] = 1/(H*L) * sum_h (sum_d q[b,l,h,:]) * (sum_d k[b,l,h,:]).
Then top-6 over l per batch, softmax, weighted sum of values rows -> [B,H,D].

Sharding: batch 16 -> 2 per core across 8 cores, no cross-core communication.

Per core: per batch, q/k row-sums + per-head products on DVE while HWDGE DMAs
stream in (k split 3+1 chunks so the last reduce is short); PE-transpose +
ACT copy + small DMA rake corr into corr2[b, :] (all off-DVE so batch 0's
stretch doesn't contend with batch 1's reduces).  Tail: one MAX8/FIND_INDEX8
pair for both batches, per-batch indirect gathers fed straight from the
FIND_INDEX8 output row (batch base via element_offset), softmax weights
stream-transposed once, per-batch tiny matmuls, stores.
"""

import numpy as np

import concourse.bass as bass
import concourse.mybir as mybir
import concourse.tile as tile
from concourse.masks import make_identity
from concourse.bass_utils import run_bass_kernel_spmd

B, L, H, D = 16, 512, 8, 64
HD = H * D                  # 512
NCORES = 8
BPC = B // NCORES           # 2 batches per core
ROWS = BPC * L              # 1024 rows of [HD] per core
P = 128
NT = ROWS // P              # 8 chunks of 128 rows
TPB = L // P                # 4 chunks per batch
KTOP = 6                    # k = int(log(512)) = 6
SCALE = 1.0 / (H * L)

_CACHE = {}


def _emit(tc, q, k, v, out):
    # out: single [BPC, HD] DRAM AP.
    nc = tc.nc
    from contextlib import ExitStack

    with ExitStack() as ctx:
        main = ctx.enter_context(tc.tile_pool(name="main", bufs=2))
        small = ctx.enter_context(tc.tile_pool(name="small", bufs=1))
        psum = ctx.enter_context(tc.tile_pool(name="psum", bufs=1, space="PSUM"))

        ident = small.tile([P, P], mybir.dt.float32)
        make_identity(nc, ident[:])

        # Per-partition batch masks m0=[1,0], m1=[0,1] (built once, early).
        m1i = small.tile([BPC, 1], mybir.dt.int32)
        nc.gpsimd.iota(m1i[:], pattern=[[0, 1]], base=0, channel_multiplier=1)
        m0i = small.tile([BPC, 1], mybir.dt.int32)
        nc.gpsimd.iota(m0i[:], pattern=[[0, 1]], base=1, channel_multiplier=-1)
        m1f = small.tile([BPC, 1], mybir.dt.float32)
        nc.vector.tensor_copy(m1f[:], m1i[:])
        m0f = small.tile([BPC, 1], mybir.dt.float32)
        nc.vector.tensor_copy(m0f[:], m0i[:])

        q3 = q.rearrange("(t p) m -> t p m", p=P)
        k3 = k.rearrange("(t p) m -> t p m", p=P)

        corr2 = small.tile([BPC, L], mybir.dt.float32)
        for b in range(BPC):
            t0 = b * TPB

            qt = main.tile([P, TPB, HD], mybir.dt.float32, tag=f"qt{b}")
            nc.sync.dma_start(
                out=qt[:], in_=q3[t0 : t0 + TPB].rearrange("t p m -> p t m")
            )
            kt = main.tile([P, TPB, HD], mybir.dt.float32, tag=f"kt{b}")
            nc.sync.dma_start(
                out=kt[:, 0:3, :],
                in_=k3[t0 : t0 + 3].rearrange("t p m -> p t m"),
            )
            nc.sync.dma_start(
                out=kt[:, 3:4, :],
                in_=k3[t0 + 3 : t0 + TPB].rearrange("t p m -> p t m"),
            )
            sq = small.tile([P, TPB * H], mybir.dt.float32, tag=f"sq{b}")
            sk = small.tile([P, TPB * H], mybir.dt.float32, tag=f"sk{b}")
            nc.vector.reduce_sum(
                out=sq[:],
                in_=qt[:].rearrange("p t (h d) -> p (t h) d", d=D),
                axis=mybir.AxisListType.X,
            )
            nc.vector.reduce_sum(
                out=sk[:, 0 : 3 * H],
                in_=kt[:, 0:3, :].rearrange("p t (h d) -> p (t h) d", d=D),
                axis=mybir.AxisListType.X,
            )
            nc.vector.reduce_sum(
                out=sk[:, 3 * H : 4 * H],
                in_=kt[:, 3:4, :].rearrange("p t (h d) -> p (t h) d", d=D),
                axis=mybir.AxisListType.X,
            )
            prod = small.tile([P, TPB * H], mybir.dt.float32, tag=f"prod{b}")
            nc.vector.tensor_mul(prod[:], sq[:], sk[:])
            corr_b = small.tile([P, TPB], mybir.dt.float32, tag=f"corr{b}")
            nc.vector.reduce_sum(
                out=corr_b[:],
                in_=prod[:].rearrange("p (t h) -> p t h", h=H),
                axis=mybir.AxisListType.X,
            )

            # corr [128, 4] -> [4, 128] on PE, ACT copies it out of PSUM,
            # one small DMA rakes it into corr2[b, :].
            psumT = psum.tile([TPB, P], mybir.dt.float32, tag=f"psumT{b}")
            nc.tensor.transpose(out=psumT[:], in_=corr_b[:], identity=ident[:])
            corrT = small.tile([TPB, P], mybir.dt.float32, tag=f"corrT{b}")
            nc.scalar.copy(corrT[:], psumT[:])
            nc.sync.dma_start(out=corr2[b : b + 1, :], in_=corrT[:])

        # ---- tail ----
        maxv = small.tile([BPC, 8], mybir.dt.float32)
        maxi = small.tile([BPC, 8], mybir.dt.uint32)
        nc.vector.max(out=maxv[:], in_=corr2[:])
        nc.vector.max_index(out=maxi[:], in_max=maxv[:], in_values=corr2[:])

        # Indices hop onto partitions via one 32x32 stream transpose
        # (uint32 end-to-end): stageT_i[0:6, b] = top-6 indices of batch b.
        stage_i = small.tile([32, 32], mybir.dt.uint32)
        nc.vector.memset(stage_i[:], 0)
        nc.vector.tensor_copy(stage_i[0:BPC, 0:KTOP], maxi[:, 0:KTOP])
        stageT_i = small.tile([32, 32], mybir.dt.uint32)
        nc.vector.transpose(out=stageT_i[:], in_=stage_i[:])

        # softmax over the top-6 of corr*SCALE (|corr*SCALE| < ~1, so
        # skipping the max-subtraction is safe in fp32); weights hop onto
        # partitions via one 32x32 stream transpose.
        e = small.tile([BPC, KTOP], mybir.dt.float32)
        nc.scalar.activation(
            out=e[:],
            in_=maxv[:, 0:KTOP],
            func=mybir.ActivationFunctionType.Exp,
            scale=SCALE,
        )
        s = small.tile([BPC, 1], mybir.dt.float32)
        nc.vector.reduce_sum(out=s[:], in_=e[:], axis=mybir.AxisListType.X)
        rs = small.tile([BPC, 1], mybir.dt.float32)
        nc.vector.reciprocal(out=rs[:], in_=s[:])
        w = small.tile([BPC, KTOP], mybir.dt.float32)
        nc.vector.tensor_scalar_mul(w[:], e[:], rs[:, 0:1])

        # Block-diagonal [12, 2] weights via the mask trick: stage_w[b, 0:6]
        # = w_b0 masked to row 0, stage_w[b, 6:12] = w_b1 masked to row 1;
        # transpose -> stageT_w[0:6, 0] = w0, stageT_w[6:12, 1] = w1.
        stage_w = small.tile([32, 32], mybir.dt.float32)
        nc.vector.memset(stage_w[:], 0.0)
        nc.vector.tensor_scalar(
            stage_w[0:BPC, 0:KTOP],
            w[:],
            m0f[:, 0:1],
            scalar2=None,
            op0=mybir.AluOpType.mult,
        )
        nc.vector.tensor_scalar(
            stage_w[0:BPC, KTOP : 2 * KTOP],
            w[:],
            m1f[:, 0:1],
            scalar2=None,
            op0=mybir.AluOpType.mult,
        )
        stageT_w = small.tile([32, 32], mybir.dt.float32)
        nc.vector.transpose(out=stageT_w[:], in_=stage_w[:])

        # Indirect-gather the 6 selected value rows per batch into one
        # [12, 512] tile (batch base via element_offset), weighted-sum both
        # batches with ONE matmul, one copy, one store.
        gath = small.tile([2 * KTOP, HD], mybir.dt.float32)
        for b in range(BPC):
            nc.gpsimd.indirect_dma_start(
                out=gath[b * KTOP : (b + 1) * KTOP, :],
                out_offset=None,
                in_=v,
                in_offset=bass.IndirectOffsetOnAxis(
                    ap=stageT_i[0:KTOP, b : b + 1], axis=0
                ),
                element_offset=b * L * HD,
            )
        acc = psum.tile([BPC, HD], mybir.dt.float32)
        nc.tensor.matmul(
            out=acc[:],
            lhsT=stageT_w[0 : 2 * KTOP, 0:BPC],
            rhs=gath[:],
            start=True,
            stop=True,
        )
        outt = small.tile([BPC, HD], mybir.dt.float32)
        nc.scalar.copy(outt[:], acc[:])
        nc.sync.dma_start(out=out, in_=outt[:])


def _build_bass():
    import concourse.bacc as bacc

    nc = bacc.Bacc(trn_type="TRN2", target_bir_lowering=False, debug=False)
    q = nc.dram_tensor("q", [ROWS, HD], mybir.dt.float32, kind="ExternalInput").ap()
    k = nc.dram_tensor("k", [ROWS, HD], mybir.dt.float32, kind="ExternalInput").ap()
    v = nc.dram_tensor("v", [ROWS, HD], mybir.dt.float32, kind="ExternalInput").ap()
    out = nc.dram_tensor(
        "out", [BPC, HD], mybir.dt.float32, kind="ExternalOutput"
    ).ap()
    with tile.TileContext(nc) as tc:
        _emit(tc, q, k, v, out)
    nc.compile()
    return nc


def _get_nc():
    if "nc" not in _CACHE:
        _CACHE["nc"] = _build_bass()
    return _CACHE["nc"]


def run_sharded(queries, keys, values, trace=False, **kw):
    """Shard over 8 cores, run, gather. Returns (out [16,8,64], BassKernelResults)."""
    nc = _get_nc()
    q = np.ascontiguousarray(np.asarray(queries, dtype=np.float32))
    k = np.ascontiguousarray(np.asarray(keys, dtype=np.float32))
    v = np.ascontiguousarray(np.asarray(values, dtype=np.float32))
    in_maps = []
    for c in range(NCORES):
        sl = slice(c * BPC, (c + 1) * BPC)
        in_maps.append(
            {
                "q": q[sl].reshape(ROWS, HD),
                "k": k[sl].reshape(ROWS, HD),
                "v": v[sl].reshape(ROWS, HD),
            }
        )
    res = run_bass_kernel_spmd(nc, in_maps, list(range(NCORES)), trace=trace, **kw)
    out = np.empty((B, H, D), dtype=np.float32)
    for c in range(NCORES):
        out[c * BPC : (c + 1) * BPC] = res.results[c]["out"].reshape(BPC, H, D)
    return out, res


def kernel(queries, keys, values, B=None, **_ignored):
    out, _ = run_sharded(queries, keys, values, trace=False)
    return out
